# revision 49
# baseline (speedup 1.0000x reference)
"""Trainium2 Bass kernel for nn_AutoregressiveDecoder (gnn_message_passing).

Math (derived from the reference):
  With Ahat = max(adj, I), CS[i,u] = sum_{v<i} Ahat[v,u], deg_i = CS[i]^-1/2,
  row i of supp reduces to
    supp[i,u<i] = 0.5*tanh(deg_i(u) * (Ahat @ (deg_i^2 * relu(Yt_i)^T r_i))[u])
    supp[i,i]   = 0.5*tanh(q_i . q_i)
  where Yt_i = Z1^T (D_i Ahat), Z1 = z @ W1[:128],
    q_i = relu(Z1[i] + W1[128]) @ W2,  r_i = W2 @ q_i.
  Output = 0.5 z z^T + supp + supp^T.

Mapping (per core, rows i = c, c+8, ..., c+248; SPMD via one-hot inputs):
  - Rows processed in 11 groups of g rows (g in {8,4,2}) sharing one PSUM
    tile: stage_a emits 2*nvb matmuls of free-size g*mjg (amortizes the
    ~180ns isolated-matmul cost), relu'd to bf16 (split ACT/DVE).
  - t_j = relu(Yt_j)^T r_j uses a diagonal-expanded rc (lhsT = 32-col
    window with only col j nonzero -> LDWEIGHTS is 32 cols); out row j of a
    persistent [32,256] PSUM tile accumulates across all rows.
  - deg via quake-rsqrt on DVE (int shift/sub + 1 Newton step): the kernel
    then needs only one ACT table set (relu/tanh/copy) -> 1 table load.
  - U-fold (Ahat @ sprime), deg scale, tanh, diag term and the 0.5 z z^T
    column strip are computed once at the end; one [128,128] f32 DMA out.
Host glue: builds per-core one-hots/masks and pre-transposed bf16 layouts,
gathers 8x[256,64] strips, returns x + supp + supp.T.
"""

import numpy as np

N = 256
NCORES = 8
NPC = N // NCORES  # 32 rows per core

# (start_row, g) per group; mjg = 8*(start+g)
GROUPS = [(0, 8), (8, 4), (12, 4)] + [(j, 2) for j in range(16, 32, 2)]
DVE_RELU = {2, 3, 4, 5, 6}  # groups whose relu runs on Vector instead of Scalar


def _s_offsets():
    """Column offsets of each group's block in the host-packed s blob.
    Groups with K<=128 store [p, q*mjg+u]; K=256 groups store the DoubleRow
    pair layout [p, ko*gm + q*mjg+u] (ko = K-half)."""
    offs = {}
    off = 0
    for gi, (j0, g) in enumerate(GROUPS):
        mjg = 8 * (j0 + g)
        nk = 1 if mjg <= 128 else 2
        offs[gi] = off
        off += nk * g * mjg
    return offs, off


S_OFFS, S_COLS = _s_offsets()  # S_COLS == 7808
DR_TCOL_MIN_ROW = 12  # rows >= this use a single DoubleRow t-matvec

_PROGRAM = None
LAST_RESULTS = None
TRACE = False
TRACE_KW = {}

QUAKE_MAGIC = 0x5F3759DF


def _build_program():
    import concourse.bacc as bacc
    import concourse.mybir as mybir
    from concourse import tile

    F32 = mybir.dt.float32
    BF16 = mybir.dt.bfloat16
    FP8 = mybir.dt.float8e4
    I32 = mybir.dt.int32
    AF = mybir.ActivationFunctionType
    ALU = mybir.AluOpType
    DR = mybir.MatmulPerfMode.DoubleRow

    nc = bacc.Bacc()

    b1_d = nc.dram_tensor("b1", [128, 1024], FP8, kind="ExternalInput")
    b2_d = nc.dram_tensor("b2", [128, 1600], BF16, kind="ExternalInput")
    b3_d = nc.dram_tensor("b3", [128, S_COLS], FP8, kind="ExternalInput")
    out_d = nc.dram_tensor("outp", [128, 128], F32, kind="ExternalOutput")

    with tile.TileContext(nc) as tc, tc.tile_pool(name="persist", bufs=1) as P:
        b1 = P.tile([128, 2, 512], FP8, tag="b1", name="b1")
        b2 = P.tile([128, 1600], BF16, tag="b2", name="b2")
        b3 = P.tile([128, S_COLS], FP8, tag="b3", name="b3")
        nc.sync.dma_start(b1[:], b1_d[:].rearrange("p (k c) -> p k c", k=2))
        nc.sync.dma_start(b2[:], b2_d[:])
        # s blob streamed in consumption order (3 chunks overlap the preamble)
        nc.sync.dma_start(b3[:, 0:1408], b3_d[:, 0:1408])
        nc.sync.dma_start(b3[:, 1408:4096], b3_d[:, 1408:4096])
        nc.sync.dma_start(b3[:, 4096:S_COLS], b3_d[:, 4096:S_COLS])
        utdr = b1[:, :, 0:256]    # [p, ko, i] = UT[ko*128+p, i]
        ahbdr = b1[:, :, 256:512]  # [p, ko, u] = Ahat[ko*128+p, u]
        ztb = b2[:, 0:256]
        w1ab = b2[:, 256:512]
        w1bb = b2[0:1, 512:768]
        w2h = [b2[:, 768:896], b2[:, 896:1024]]
        w2tb = b2[:, 1024:1280]
        ocb = [b2[:, 1280:1312], b2[:, 1312:1344]]
        mcb = b2[:, 1344:1408]                      # [128, 2*32]
        identb = b2[:, 1408:1536]
        zoc = b2[:, 1536:1568]                      # z rows for this core, [128d, 32]

        wsb = P.tile([128, 512], BF16, tag="wsb", name="wsb")
        nc.vector.memset(wsb[:], 0.0)
        onesb = P.tile([1, 256], BF16, tag="onesb", name="onesb")
        nc.vector.memset(onesb[:], 1.0)
        onescol = P.tile([128, 1], BF16, tag="onescol", name="onescol")
        nc.vector.memset(onescol[:], 1.0)
        zero32 = P.tile([128, 32], BF16, tag="zero32", name="zero32")
        nc.vector.memset(zero32[:], 0.0)
        magic = P.tile([128, 64], I32, tag="magic", name="magic")
        nc.vector.memset(magic[:], QUAKE_MAGIC)
        rczdr = P.tile([128, 2, 1056], FP8, tag="rczdr", name="rczdr")
        nc.vector.memset(rczdr[:], 0.0)

        # persistent SBUF intermediates
        cs_sb = P.tile([128, 2, 256], BF16, tag="cs_sb", name="cs_sb")
        z1dr = P.tile([128, 2, 256], FP8, tag="z1dr", name="z1dr")
        rbt = [P.tile([128, 256], BF16, tag=f"rbt{hb}", name=f"rbt{hb}") for hb in range(2)]
        qtb = P.tile([128, 256], BF16, tag="qtb", name="qtb")
        sqb = P.tile([128, 256], BF16, tag="sqb", name="sqb")
        rsb = [P.tile([128, 256], BF16, tag=f"rsb{nb}", name=f"rsb{nb}") for nb in range(2)]
        rc_sb = P.tile([128, 2, 32], BF16, tag="rc_sb", name="rc_sb")
        csc_sb = P.tile([128, 64], F32, tag="csc_sb", name="csc_sb")
        qi32 = P.tile([128, 64], I32, tag="qi32", name="qi32")
        y0 = P.tile([128, 64], F32, tag="y0", name="y0")
        yt2 = P.tile([128, 64], F32, tag="yt2", name="yt2")
        mcf = P.tile([128, 64], F32, tag="mcf", name="mcf")
        degf = P.tile([128, 64], F32, tag="degf", name="degf")
        degcb = P.tile([128, 2, 32], BF16, tag="degcb", name="degcb")
        degcT = P.tile([32, 256], BF16, tag="degcT", name="degcT")
        ddctT = P.tile([32, 256], F32, tag="ddctT", name="ddctT")
        tqh = P.tile([128, 2], F32, tag="tqh", name="tqh")
        dg = P.tile([128, 2, 32], F32, tag="dg", name="dg")
        fin = P.tile([128, 2, 64], F32, tag="fin", name="fin")
        spT = P.tile([32, 256], BF16, tag="spT", name="spT")
        spc = P.tile([128, 2, 32], FP8, tag="spc", name="spc")
        wt = P.tile([128, 64], F32, tag="wt", name="wt")
        tht = P.tile([128, 64], F32, tag="tht", name="tht")

        with tc.tile_pool(name="pre_ps", bufs=2, space="PSUM") as PS:
            # warm the PE clock (HAM) while the input DMAs are in flight:
            # wide matmuls (FD=512) keep the array genuinely busy, unlike
            # small-FD spam whose duty cycle is too low to unthrottle
            warm = PS.tile([16, 512], F32, tag="warm", name="warm")
            for _ in range(16):
                nc.tensor.matmul(warm[:], wsb[:, 0:16], wsb[:],
                                 start=True, stop=True)

            # CS[i,u] = sum_w UT[w,i] Ahat[w,u]  (DoubleRow over the 256 w's)
            for ib in range(2):
                ps = PS.tile([128, 256], F32, tag="ps", name="ps")
                nc.tensor.matmul(ps[:], utdr[:, :, ib * 128:(ib + 1) * 128],
                                 ahbdr[:, :, :], start=True, stop=True,
                                 perf_mode=DR)
                nc.scalar.activation(cs_sb[:, ib, :], ps[:], AF.Copy)

            # Z1 = z @ W1a   (lhsT = z^T block, rhs = W1a) — also covers the
            # ACT cs-copy latency before the select matmuls below
            for nb in range(2):
                ps = PS.tile([128, 256], F32, tag="ps", name="ps")
                nc.tensor.matmul(ps[:], ztb[:, nb * 128:(nb + 1) * 128], w1ab[:],
                                 start=True, stop=True)
                nc.vector.tensor_copy(z1dr[:, nb, :], ps[:])

            # CSC[u, (ub,j)] = CS[i_j, u]  (select rows of CS via one-hots)
            csc = PS.tile([128, 2, 32], F32, tag="csc", name="csc")
            for ub in range(2):
                for ib in range(2):
                    nc.tensor.matmul(csc[:, ub, :],
                                     cs_sb[:, ib, ub * 128:(ub + 1) * 128], ocb[ib][:],
                                     start=(ib == 0), stop=(ib == 1))
            nc.vector.tensor_copy(csc_sb[:], csc[:, :, :])

            # rbt = relu(W1^T [z|1]^T)
            for hb in range(2):
                ps = PS.tile([128, 256], F32, tag="ps", name="ps")
                nc.tensor.matmul(ps[:], w1ab[:, hb * 128:(hb + 1) * 128], ztb[:],
                                 start=True, stop=False)
                nc.tensor.matmul(ps[:], w1bb[:, hb * 128:(hb + 1) * 128], onesb[:],
                                 start=False, stop=True)
                nc.scalar.activation(rbt[hb][:], ps[:], AF.Relu)

            # Q^T = W2^T relu(ZB)^T  -> qtb [d, n]
            ps = PS.tile([128, 256], F32, tag="ps", name="ps")
            for hb in range(2):
                nc.tensor.matmul(ps[:], w2h[hb][:], rbt[hb][:],
                                 start=(hb == 0), stop=(hb == 1))
            nc.vector.tensor_copy(qtb[:], ps[:])
            nc.vector.tensor_mul(sqb[:], qtb[:], qtb[:])

            # R = Q @ W2^T  -> rsb [n-block, h]
            for nb in range(2):
                ps = PS.tile([128, 256], F32, tag="ps", name="ps")
                nc.tensor.matmul(ps[:], qtb[:, nb * 128:(nb + 1) * 128], w2tb[:],
                                 start=True, stop=True)
                nc.vector.tensor_copy(rsb[nb][:], ps[:])

            # rc[h, j] = R[i_j, h]
            rcps = PS.tile([128, 2, 32], F32, tag="csc", name="csc")
            for hb in range(2):
                for nb in range(2):
                    nc.tensor.matmul(rcps[:, hb, :],
                                     rsb[nb][:, hb * 128:(hb + 1) * 128], ocb[nb][:],
                                     start=(nb == 0), stop=(nb == 1))
            nc.vector.tensor_copy(rc_sb[:, :, :], rcps[:, :, :])
            # diagonal-expand rc into rczdr (col j of window j nonzero)
            for hb in range(2):
                dst = rczdr[:, hb, :].rearrange("p (j k) -> p j k", k=33)[:, :, 0:1]
                nc.vector.tensor_copy(dst, rc_sb[:, hb, :].unsqueeze(2))

            # quake rsqrt: deg = CS^-1/2 (exact-int CS; 1 Newton step) — feeds
            # only the tail, so it sits after the loop-critical DVE ops
            nc.vector.tensor_single_scalar(qi32[:], csc_sb[:].bitcast(I32), 1,
                                           ALU.arith_shift_right)
            nc.vector.tensor_sub(y0[:].bitcast(I32), magic[:], qi32[:])
            nc.vector.tensor_mul(yt2[:], y0[:], y0[:])
            nc.vector.tensor_mul(yt2[:], yt2[:], csc_sb[:])
            nc.vector.tensor_scalar(yt2[:], yt2[:], -0.5, 1.5, ALU.mult, ALU.add)
            nc.vector.tensor_mul(degf[:], y0[:], yt2[:])
            nc.vector.tensor_copy(mcf[:], mcb[:])
            nc.vector.tensor_mul(degf[:], degf[:], mcf[:])
            nc.vector.tensor_copy(degcb[:, :, :],
                                  degf[:].rearrange("p (u j) -> p u j", u=2))

            # qq[n] = |q_n|^2 ; tqh = tanh(qq); dg = 0.5 * oc * tqh
            qq = PS.tile([128, 2, 32], F32, tag="csc", name="qq")
            for nb in range(2):
                nc.tensor.matmul(qq[:, nb, 0:1], sqb[:, nb * 128:(nb + 1) * 128],
                                 onescol[:], start=True, stop=True)
            nc.scalar.activation(tqh[:].rearrange("p (u j) -> p u j", u=2),
                                 qq[:, :, 0:1], AF.Tanh)
            for ib in range(2):
                nc.vector.tensor_scalar(dg[:, ib, :], ocb[ib][:], tqh[:, ib:ib + 1],
                                        0.5, ALU.mult, ALU.mult)

            # X strip: 0.5 * z z^T columns for this core
            for ub in range(2):
                ps = PS.tile([128, 2, 32], F32, tag="csc", name="csc")
                nc.tensor.matmul(ps[:, 0, :], ztb[:, ub * 128:(ub + 1) * 128], zoc[:],
                                 start=True, stop=True)
                nc.vector.tensor_scalar_mul(fin[:, ub, 32:64], ps[:, 0, :], 0.5)

            # degcT[j, u] = degc[u, j]; ddctT = degcT^2
            for ub in range(2):
                pst = PS.tile([32, 128], BF16, tag="pst", name="pst")
                nc.tensor.transpose(pst[:], degcb[:, ub, :], identb[:])
                nc.vector.tensor_copy(degcT[:, ub * 128:(ub + 1) * 128], pst[:])
            nc.vector.tensor_mul(ddctT[:], degcT[:], degcT[:])

        # ---------------- grouped row loop ----------------
        with tc.tile_pool(name="tp", bufs=1, space="PSUM") as TP:
            t_rows = TP.tile([128, 256], F32, tag="t_rows", name="t_rows")
            # clear has_written across [0:32, 0:256] (zero weights)
            nc.tensor.matmul(t_rows[0:32, 0:256], zero32[:], wsb[:, 0:256],
                             start=True, stop=False, skip_group_check=True)

            with tc.tile_pool(name="loop_ps", bufs=3, space="PSUM") as LPS, \
                 tc.tile_pool(name="loop_sb", bufs=4) as LSB:

                def stage_a(gi):
                    j0, g = GROUPS[gi]
                    mjg = 8 * (j0 + g)
                    gm = g * mjg
                    off = S_OFFS[gi]
                    yt = LPS.tile([128, 2, 512], F32, tag="yt", name="yt")
                    for hb in range(2):
                        if mjg <= 128:
                            # plain fp8 matmul, K = mjg on the low z1 half
                            nc.tensor.matmul(yt[:, hb, 0:gm],
                                             z1dr[0:mjg, 0, hb * 128:(hb + 1) * 128],
                                             b3[0:mjg, off:off + gm],
                                             start=True, stop=True)
                        else:
                            # DoubleRow: K=256 packed as (w, w+128) pairs
                            rhs = b3[:, off:off + 2 * gm].rearrange(
                                "p (k n) -> p k n", k=2)
                            nc.tensor.matmul(yt[:, hb, 0:gm],
                                             z1dr[:, :, hb * 128:(hb + 1) * 128],
                                             rhs, start=True, stop=True,
                                             perf_mode=DR)
                    return (gi, j0, g, mjg, yt)

                def stage_b(state):
                    gi, j0, g, mjg, yt = state
                    gm = g * mjg
                    last = (gi == len(GROUPS) - 1)
                    ftt = LSB.tile([128, 2, 512], FP8, tag="ftt", name="ftt")
                    if gi >= 9:
                        # drain-critical last groups: split relu across engines
                        nc.scalar.activation(ftt[:, 0, 0:gm], yt[:, 0, 0:gm], AF.Relu)
                        nc.vector.tensor_scalar_max(ftt[:, 1, 0:gm], yt[:, 1, 0:gm], 0.0)
                    elif gi in DVE_RELU:
                        nc.vector.tensor_scalar_max(ftt[:, :, 0:gm], yt[:, :, 0:gm], 0.0)
                    else:
                        nc.scalar.activation(ftt[:, :, 0:gm], yt[:, :, 0:gm], AF.Relu)
                    for q in range(g):
                        j = j0 + q
                        stop = last and q == g - 1
                        if j >= DR_TCOL_MIN_ROW:
                            # one DoubleRow t-matvec: K=256 as (h, h+128) pairs
                            nc.tensor.matmul(
                                t_rows[0:32, 0:mjg],
                                rczdr[:, :, j * 32:j * 32 + 32],
                                ftt[:, :, q * mjg:(q + 1) * mjg],
                                start=False, stop=stop,
                                perf_mode=DR, skip_group_check=True)
                        else:
                            for hb in range(2):
                                nc.tensor.matmul(
                                    t_rows[0:32, 0:mjg],
                                    rczdr[:, hb, j * 32:j * 32 + 32],
                                    ftt[:, hb, q * mjg:(q + 1) * mjg],
                                    start=False, stop=(stop and hb == 1),
                                    skip_group_check=True)

                pend = []
                for gi in range(len(GROUPS)):
                    st = stage_a(gi)
                    pend.append(st)
                    if len(pend) > 2:
                        stage_b(pend.pop(0))
                for st in pend:
                    stage_b(st)

            # ---------------- tail (t_rows pool still open) ----------------
            with tc.tile_pool(name="tail_ps", bufs=2, space="PSUM") as TPS:
                nc.vector.tensor_mul(spT[:], t_rows[0:32, :], ddctT[:])
                for vb in range(2):
                    pst2 = TPS.tile([128, 32], BF16, tag="pst2", name="pst2")
                    nc.tensor.transpose(pst2[:], spT[:, vb * 128:(vb + 1) * 128],
                                        identb[0:32, 0:32])
                    nc.vector.tensor_copy(spc[:, vb, :], pst2[:])
                up = TPS.tile([128, 2, 32], F32, tag="up", name="up")
                for ub in range(2):
                    nc.tensor.matmul(up[:, ub, :],
                                     ahbdr[:, :, ub * 128:(ub + 1) * 128],
                                     spc[:, :, :],
                                     start=True, stop=True, perf_mode=DR)
                nc.vector.tensor_mul(wt[:], up[:, :, :].rearrange("p u j -> p (u j)"),
                                     degf[:])
                nc.scalar.activation(tht[:], wt[:], AF.Tanh)
                # fin = 0.5 * tanh + dg  (the 0.5 scales the tanh OUTPUT)
                nc.vector.scalar_tensor_tensor(
                    fin[:, :, 0:32],
                    tht[:].rearrange("p (u j) -> p u j", u=2),
                    0.5, dg[:, :, :], ALU.mult, ALU.add)
        nc.sync.dma_start(out_d[:], fin[:, :, :].rearrange("p u j -> p (u j)"))

    nc.finalize()
    return nc


def _get_program():
    global _PROGRAM
    if _PROGRAM is None:
        _PROGRAM = _build_program()
    return _PROGRAM


def kernel(z, adj, W1, W2):
    global LAST_RESULTS
    from concourse.bass_utils import run_bass_kernel_spmd
    import ml_dtypes

    bf = ml_dtypes.bfloat16
    z = np.asarray(z, np.float32)
    adj = np.asarray(adj, np.float32)
    W1 = np.asarray(W1, np.float32)
    W2 = np.asarray(W2, np.float32)

    idx = np.arange(N)
    Ahat = np.maximum(adj, np.eye(N, dtype=np.float32))
    UT = (idx[:, None] < idx[None, :]).astype(np.float32)
    zt = z.T  # [128, 256]
    # column-sum prefix: CS[i, u] = sum_{v<i} Ahat[v, u]
    CSex = np.vstack([np.zeros((1, N), np.float32), np.cumsum(Ahat, axis=0)[:-1]])

    nc = _get_program()
    in_maps = []
    for c in range(NCORES):
        ii = np.arange(c, N, NCORES)
        OC = np.zeros((N, NPC), np.float32)
        OC[ii, np.arange(NPC)] = 1.0
        MC = (idx[:, None] < ii[None, :]).astype(np.float32)  # [256, 32]

        import concourse.mybir as _mybir
        f8 = _mybir.dt.np(_mybir.dt.float8e4)
        # [p, ko*512 + c]: c in 0:256 -> UT[ko*128+p], c in 256:512 -> Ahat
        b1 = np.zeros((128, 1024), f8)
        for ko in range(2):
            b1[:, ko * 512:ko * 512 + 256] = UT[ko * 128:(ko + 1) * 128].astype(f8)
            b1[:, ko * 512 + 256:ko * 512 + 512] = \
                Ahat[ko * 128:(ko + 1) * 128].astype(f8)

        b2 = np.zeros((128, 1600), bf)
        b2[:, 0:256] = zt.astype(bf)
        b2[:, 256:512] = W1[0:128].astype(bf)
        b2[0, 512:768] = W1[128].astype(bf)
        b2[:, 768:896] = W2[0:128].astype(bf)
        b2[:, 896:1024] = W2[128:256].astype(bf)
        b2[:, 1024:1280] = W2.T.astype(bf)
        b2[:, 1280:1312] = OC[0:128].astype(bf)
        b2[:, 1312:1344] = OC[128:256].astype(bf)
        b2[:, 1344:1376] = MC[0:128].astype(bf)
        b2[:, 1376:1408] = MC[128:256].astype(bf)
        b2[:, 1408:1536] = np.eye(128, dtype=np.float32).astype(bf)
        b2[:, 1536:1568] = zt[:, ii].astype(bf)

        # deg-scaled adjacency operand blocks (deg_j(w) * Ahat[w, u]) in fp8,
        # packed group-major; K=256 groups use the DoubleRow half layout
        with np.errstate(divide="ignore"):
            degM = np.where(idx[:, None] < ii[None, :],
                            CSex[ii].T ** -0.5, 0.0).astype(np.float32)  # [w, j]
        b3 = np.zeros((128, S_COLS), f8)
        for gi, (j0, g) in enumerate(GROUPS):
            mjg = 8 * (j0 + g)
            gm = g * mjg
            off = S_OFFS[gi]
            nk = 1 if mjg <= 128 else 2
            for ko in range(nk):
                w0 = ko * 128
                sz = min(mjg - w0, 128)
                blk = (Ahat[w0:w0 + sz, None, 0:mjg]
                       * degM[w0:w0 + sz, j0:j0 + g, None])  # [sz, g, mjg]
                b3[0:sz, off + ko * gm:off + ko * gm + gm] = \
                    blk.reshape(sz, gm).astype(f8)
        in_maps.append({"b1": b1, "b2": b2, "b3": b3})

    res = run_bass_kernel_spmd(nc, in_maps, list(range(NCORES)),
                               trace=TRACE, **TRACE_KW)
    LAST_RESULTS = res

    supp = np.zeros((N, N), np.float32)
    x = np.zeros((N, N), np.float32)
    for c in range(NCORES):
        ii = np.arange(c, N, NCORES)
        out_r = np.asarray(res.results[c]["outp"], np.float32).reshape(128, 2, 64)
        # supp[i_j, u] where u = ub*128 + p
        supp[ii, :] = out_r[:, :, 0:32].transpose(2, 1, 0).reshape(NPC, N)
        # x[u, i_j]
        x[:, ii] = out_r[:, :, 32:64].transpose(1, 0, 2).reshape(N, NPC)
    return (x + supp + supp.T).astype(np.float32)


# revision 51
# speedup vs baseline: 1.1876x; 1.1876x over previous
"""Trainium2 Bass kernel for nn_AutoregressiveDecoder (gnn_message_passing).

Math (derived from the reference):
  With Ahat = max(adj, I), CS[i,u] = sum_{v<i} Ahat[v,u], deg_i = CS[i]^-1/2,
  row i of supp reduces to
    supp[i,u<i] = 0.5*tanh(deg_i(u) * (Ahat @ (deg_i^2 * relu(Yt_i)^T r_i))[u])
    supp[i,i]   = 0.5*tanh(q_i . q_i)
  where Yt_i = Z1^T (D_i Ahat), Z1 = z @ W1[:128],
    q_i = relu(Z1[i] + W1[128]) @ W2,  r_i = W2 @ q_i.
  Output = 0.5 z z^T + supp + supp^T.

Mapping (per core, rows i = c, c+8, ..., c+248; SPMD via one-hot inputs):
  - Rows processed in 11 groups of g rows (g in {8,4,2}) sharing one PSUM
    tile: stage_a emits 2*nvb matmuls of free-size g*mjg (amortizes the
    ~180ns isolated-matmul cost), relu'd to bf16 (split ACT/DVE).
  - t_j = relu(Yt_j)^T r_j uses a diagonal-expanded rc (lhsT = 32-col
    window with only col j nonzero -> LDWEIGHTS is 32 cols); out row j of a
    persistent [32,256] PSUM tile accumulates across all rows.
  - deg via quake-rsqrt on DVE (int shift/sub + 1 Newton step): the kernel
    then needs only one ACT table set (relu/tanh/copy) -> 1 table load.
  - U-fold (Ahat @ sprime), deg scale, tanh, diag term and the 0.5 z z^T
    column strip are computed once at the end; one [128,128] f32 DMA out.
Host glue: builds per-core one-hots/masks and pre-transposed bf16 layouts,
gathers 8x[256,64] strips, returns x + supp + supp.T.
"""

import numpy as np

N = 256
NCORES = 8
NPC = N // NCORES  # 32 rows per core

# (start_row, g) per group; mjg = 8*(start+g)
GROUPS = [(0, 8), (8, 4), (12, 4)] + [(j, 2) for j in range(16, 32, 2)]
DVE_RELU = {2, 3, 4, 5, 6}  # groups whose relu runs on Vector instead of Scalar


def _s_offsets():
    """Column offsets of each group's block in the host-packed s blob.
    Groups with K<=128 store [p, q*mjg+u]; K=256 groups store the DoubleRow
    pair layout [p, ko*gm + q*mjg+u] (ko = K-half)."""
    offs = {}
    off = 0
    for gi, (j0, g) in enumerate(GROUPS):
        mjg = 8 * (j0 + g)
        nk = 1 if mjg <= 128 else 2
        offs[gi] = off
        off += nk * g * mjg
    return offs, off


S_OFFS, S_COLS = _s_offsets()  # S_COLS == 7808
DR_TCOL_MIN_ROW = 12  # rows >= this use a single DoubleRow t-matvec

_PROGRAM = None
LAST_RESULTS = None
TRACE = False
TRACE_KW = {}

QUAKE_MAGIC = 0x5F3759DF


def _build_program():
    import concourse.bacc as bacc
    import concourse.mybir as mybir
    from concourse import tile

    F32 = mybir.dt.float32
    BF16 = mybir.dt.bfloat16
    FP8 = mybir.dt.float8e4
    I32 = mybir.dt.int32
    AF = mybir.ActivationFunctionType
    ALU = mybir.AluOpType
    DR = mybir.MatmulPerfMode.DoubleRow

    nc = bacc.Bacc()

    b1_d = nc.dram_tensor("b1", [128, 1024], FP8, kind="ExternalInput")
    b2_d = nc.dram_tensor("b2", [128, 1600], BF16, kind="ExternalInput")
    b3_d = nc.dram_tensor("b3", [128, S_COLS], FP8, kind="ExternalInput")
    out_d = nc.dram_tensor("outp", [128, 128], F32, kind="ExternalOutput")

    with tile.TileContext(nc) as tc, tc.tile_pool(name="persist", bufs=1) as P:
        b1 = P.tile([128, 2, 512], FP8, tag="b1", name="b1")
        b2 = P.tile([128, 1600], BF16, tag="b2", name="b2")
        b3 = P.tile([128, S_COLS], FP8, tag="b3", name="b3")
        nc.sync.dma_start(b1[:], b1_d[:].rearrange("p (k c) -> p k c", k=2))
        nc.sync.dma_start(b2[:], b2_d[:])
        # s blob streamed in consumption order (3 chunks overlap the preamble)
        nc.sync.dma_start(b3[:, 0:1408], b3_d[:, 0:1408])
        nc.sync.dma_start(b3[:, 1408:4096], b3_d[:, 1408:4096])
        nc.sync.dma_start(b3[:, 4096:S_COLS], b3_d[:, 4096:S_COLS])
        utdr = b1[:, :, 0:256]    # [p, ko, i] = UT[ko*128+p, i]
        ahbdr = b1[:, :, 256:512]  # [p, ko, u] = Ahat[ko*128+p, u]
        ztb = b2[:, 0:256]
        w1ab = b2[:, 256:512]
        w1bb = b2[0:1, 512:768]
        w2h = [b2[:, 768:896], b2[:, 896:1024]]
        w2tb = b2[:, 1024:1280]
        ocb = [b2[:, 1280:1312], b2[:, 1312:1344]]
        mcb = b2[:, 1344:1408]                      # [128, 2*32]
        identb = b2[:, 1408:1536]
        zoc = b2[:, 1536:1568]                      # z rows for this core, [128d, 32]

        wsb = P.tile([128, 512], BF16, tag="wsb", name="wsb")
        nc.vector.memset(wsb[:], 0.0)
        onesb = P.tile([1, 256], BF16, tag="onesb", name="onesb")
        nc.vector.memset(onesb[:], 1.0)
        onescol = P.tile([128, 1], BF16, tag="onescol", name="onescol")
        nc.vector.memset(onescol[:], 1.0)
        zero32 = P.tile([128, 32], BF16, tag="zero32", name="zero32")
        nc.vector.memset(zero32[:], 0.0)
        magic = P.tile([128, 64], I32, tag="magic", name="magic")
        nc.vector.memset(magic[:], QUAKE_MAGIC)
        rczdr = P.tile([128, 2, 1056], FP8, tag="rczdr", name="rczdr")
        nc.vector.memset(rczdr[:], 0.0)

        # persistent SBUF intermediates
        cs_sb = P.tile([128, 2, 256], BF16, tag="cs_sb", name="cs_sb")
        z1dr = P.tile([128, 2, 256], FP8, tag="z1dr", name="z1dr")
        rbt = [P.tile([128, 256], BF16, tag=f"rbt{hb}", name=f"rbt{hb}") for hb in range(2)]
        qtb = P.tile([128, 256], BF16, tag="qtb", name="qtb")
        sqb = P.tile([128, 256], BF16, tag="sqb", name="sqb")
        rsb = [P.tile([128, 256], BF16, tag=f"rsb{nb}", name=f"rsb{nb}") for nb in range(2)]
        rc_sb = P.tile([128, 2, 32], BF16, tag="rc_sb", name="rc_sb")
        csc_sb = P.tile([128, 64], F32, tag="csc_sb", name="csc_sb")
        qi32 = P.tile([128, 64], I32, tag="qi32", name="qi32")
        y0 = P.tile([128, 64], F32, tag="y0", name="y0")
        yt2 = P.tile([128, 64], F32, tag="yt2", name="yt2")
        mcf = P.tile([128, 64], F32, tag="mcf", name="mcf")
        degf = P.tile([128, 64], F32, tag="degf", name="degf")
        degcb = P.tile([128, 2, 32], BF16, tag="degcb", name="degcb")
        degcT = P.tile([32, 256], BF16, tag="degcT", name="degcT")
        ddctT = P.tile([32, 256], F32, tag="ddctT", name="ddctT")
        tqh = P.tile([128, 2], F32, tag="tqh", name="tqh")
        dg = P.tile([128, 2, 32], F32, tag="dg", name="dg")
        fin = P.tile([128, 2, 64], F32, tag="fin", name="fin")
        spT = P.tile([32, 256], BF16, tag="spT", name="spT")
        spc = P.tile([128, 2, 32], FP8, tag="spc", name="spc")
        wt = P.tile([128, 64], F32, tag="wt", name="wt")
        tht = P.tile([128, 64], F32, tag="tht", name="tht")

        with tc.tile_pool(name="pre_ps", bufs=2, space="PSUM") as PS:
            # warm the PE clock (HAM) while the input DMAs are in flight:
            # wide matmuls (FD=512) keep the array genuinely busy, unlike
            # small-FD spam whose duty cycle is too low to unthrottle
            warm = PS.tile([16, 512], F32, tag="warm", name="warm")
            for _ in range(16):
                nc.tensor.matmul(warm[:], wsb[:, 0:16], wsb[:],
                                 start=True, stop=True)

            # CS[i,u] = sum_w UT[w,i] Ahat[w,u]  (DoubleRow over the 256 w's)
            for ib in range(2):
                ps = PS.tile([128, 256], F32, tag="ps", name="ps")
                nc.tensor.matmul(ps[:], utdr[:, :, ib * 128:(ib + 1) * 128],
                                 ahbdr[:, :, :], start=True, stop=True,
                                 perf_mode=DR)
                nc.scalar.activation(cs_sb[:, ib, :], ps[:], AF.Copy)

            # Z1 = z @ W1a   (lhsT = z^T block, rhs = W1a) — also covers the
            # ACT cs-copy latency before the select matmuls below
            for nb in range(2):
                ps = PS.tile([128, 256], F32, tag="ps", name="ps")
                nc.tensor.matmul(ps[:], ztb[:, nb * 128:(nb + 1) * 128], w1ab[:],
                                 start=True, stop=True)
                nc.vector.tensor_copy(z1dr[:, nb, :], ps[:])

            # CSC[u, (ub,j)] = CS[i_j, u]  (select rows of CS via one-hots)
            csc = PS.tile([128, 2, 32], F32, tag="csc", name="csc")
            for ub in range(2):
                for ib in range(2):
                    nc.tensor.matmul(csc[:, ub, :],
                                     cs_sb[:, ib, ub * 128:(ub + 1) * 128], ocb[ib][:],
                                     start=(ib == 0), stop=(ib == 1))
            nc.vector.tensor_copy(csc_sb[:], csc[:, :, :])

            # rbt = relu(W1^T [z|1]^T)
            for hb in range(2):
                ps = PS.tile([128, 256], F32, tag="ps", name="ps")
                nc.tensor.matmul(ps[:], w1ab[:, hb * 128:(hb + 1) * 128], ztb[:],
                                 start=True, stop=False)
                nc.tensor.matmul(ps[:], w1bb[:, hb * 128:(hb + 1) * 128], onesb[:],
                                 start=False, stop=True)
                nc.scalar.activation(rbt[hb][:], ps[:], AF.Relu)

            # Q^T = W2^T relu(ZB)^T  -> qtb [d, n]
            ps = PS.tile([128, 256], F32, tag="ps", name="ps")
            for hb in range(2):
                nc.tensor.matmul(ps[:], w2h[hb][:], rbt[hb][:],
                                 start=(hb == 0), stop=(hb == 1))
            nc.vector.tensor_copy(qtb[:], ps[:])
            nc.vector.tensor_mul(sqb[:], qtb[:], qtb[:])

            # R = Q @ W2^T  -> rsb [n-block, h]
            for nb in range(2):
                ps = PS.tile([128, 256], F32, tag="ps", name="ps")
                nc.tensor.matmul(ps[:], qtb[:, nb * 128:(nb + 1) * 128], w2tb[:],
                                 start=True, stop=True)
                nc.vector.tensor_copy(rsb[nb][:], ps[:])

            # rc[h, j] = R[i_j, h]
            rcps = PS.tile([128, 2, 32], F32, tag="csc", name="csc")
            for hb in range(2):
                for nb in range(2):
                    nc.tensor.matmul(rcps[:, hb, :],
                                     rsb[nb][:, hb * 128:(hb + 1) * 128], ocb[nb][:],
                                     start=(nb == 0), stop=(nb == 1))
            nc.vector.tensor_copy(rc_sb[:, :, :], rcps[:, :, :])
            # diagonal-expand rc into rczdr (col j of window j nonzero)
            for hb in range(2):
                dst = rczdr[:, hb, :].rearrange("p (j k) -> p j k", k=33)[:, :, 0:1]
                nc.vector.tensor_copy(dst, rc_sb[:, hb, :].unsqueeze(2))

            # quake rsqrt: deg = CS^-1/2 (exact-int CS; 1 Newton step)
            nc.vector.tensor_single_scalar(qi32[:], csc_sb[:].bitcast(I32), 1,
                                           ALU.arith_shift_right)
            nc.vector.tensor_sub(y0[:].bitcast(I32), magic[:], qi32[:])
            nc.vector.tensor_mul(yt2[:], y0[:], y0[:])
            nc.vector.tensor_mul(yt2[:], yt2[:], csc_sb[:])
            nc.vector.tensor_scalar(yt2[:], yt2[:], -0.5, 1.5, ALU.mult, ALU.add)
            nc.vector.tensor_mul(degf[:], y0[:], yt2[:])
            nc.vector.tensor_copy(mcf[:], mcb[:])
            nc.vector.tensor_mul(degf[:], degf[:], mcf[:])
            nc.vector.tensor_copy(degcb[:, :, :],
                                  degf[:].rearrange("p (u j) -> p u j", u=2))
            # transpose deg early so the tail only multiplies: the scheduler
            # places these PE ops by dependency, so keep deps short here
            for ub in range(2):
                pstq = PS.tile([32, 128], BF16, tag="pst", name="pstq")
                nc.tensor.transpose(pstq[:], degcb[:, ub, :], identb[:])
                nc.vector.tensor_copy(degcT[:, ub * 128:(ub + 1) * 128], pstq[:])
            nc.vector.tensor_mul(ddctT[:], degcT[:], degcT[:])

            # qq[n] = |q_n|^2 ; tqh = tanh(qq); dg = 0.5 * oc * tqh
            qq = PS.tile([128, 2, 32], F32, tag="csc", name="qq")
            for nb in range(2):
                nc.tensor.matmul(qq[:, nb, 0:1], sqb[:, nb * 128:(nb + 1) * 128],
                                 onescol[:], start=True, stop=True)
            nc.scalar.activation(tqh[:].rearrange("p (u j) -> p u j", u=2),
                                 qq[:, :, 0:1], AF.Tanh)
            for ib in range(2):
                nc.vector.tensor_scalar(dg[:, ib, :], ocb[ib][:], tqh[:, ib:ib + 1],
                                        0.5, ALU.mult, ALU.mult)

            # X strip: 0.5 * z z^T columns for this core
            for ub in range(2):
                ps = PS.tile([128, 2, 32], F32, tag="csc", name="csc")
                nc.tensor.matmul(ps[:, 0, :], ztb[:, ub * 128:(ub + 1) * 128], zoc[:],
                                 start=True, stop=True)
                nc.vector.tensor_scalar_mul(fin[:, ub, 32:64], ps[:, 0, :], 0.5)

        # ---------------- grouped row loop ----------------
        with tc.tile_pool(name="tp", bufs=1, space="PSUM") as TP:
            t_rows = TP.tile([128, 256], F32, tag="t_rows", name="t_rows")
            # clear has_written across [0:32, 0:256] (zero weights)
            nc.tensor.matmul(t_rows[0:32, 0:256], zero32[:], wsb[:, 0:256],
                             start=True, stop=False, skip_group_check=True)

            with tc.tile_pool(name="loop_ps", bufs=3, space="PSUM") as LPS, \
                 tc.tile_pool(name="loop_sb", bufs=4) as LSB:

                def stage_a(gi):
                    j0, g = GROUPS[gi]
                    mjg = 8 * (j0 + g)
                    gm = g * mjg
                    off = S_OFFS[gi]
                    yt = LPS.tile([128, 2, 512], F32, tag="yt", name="yt")
                    for hb in range(2):
                        if mjg <= 128:
                            # plain fp8 matmul, K = mjg on the low z1 half
                            nc.tensor.matmul(yt[:, hb, 0:gm],
                                             z1dr[0:mjg, 0, hb * 128:(hb + 1) * 128],
                                             b3[0:mjg, off:off + gm],
                                             start=True, stop=True)
                        else:
                            # DoubleRow: K=256 packed as (w, w+128) pairs
                            rhs = b3[:, off:off + 2 * gm].rearrange(
                                "p (k n) -> p k n", k=2)
                            nc.tensor.matmul(yt[:, hb, 0:gm],
                                             z1dr[:, :, hb * 128:(hb + 1) * 128],
                                             rhs, start=True, stop=True,
                                             perf_mode=DR)
                    return (gi, j0, g, mjg, yt)

                def stage_b(state):
                    gi, j0, g, mjg, yt = state
                    gm = g * mjg
                    last = (gi == len(GROUPS) - 1)
                    ftt = LSB.tile([128, 2, 512], FP8, tag="ftt", name="ftt")
                    if gi >= 9:
                        # drain-critical last groups: split relu across engines
                        nc.scalar.activation(ftt[:, 0, 0:gm], yt[:, 0, 0:gm], AF.Relu)
                        nc.vector.tensor_scalar_max(ftt[:, 1, 0:gm], yt[:, 1, 0:gm], 0.0)
                    elif gi in DVE_RELU:
                        nc.vector.tensor_scalar_max(ftt[:, :, 0:gm], yt[:, :, 0:gm], 0.0)
                    else:
                        nc.scalar.activation(ftt[:, :, 0:gm], yt[:, :, 0:gm], AF.Relu)
                    for q in range(g):
                        j = j0 + q
                        stop = last and q == g - 1
                        if j >= DR_TCOL_MIN_ROW:
                            # one DoubleRow t-matvec: K=256 as (h, h+128) pairs
                            nc.tensor.matmul(
                                t_rows[0:32, 0:mjg],
                                rczdr[:, :, j * 32:j * 32 + 32],
                                ftt[:, :, q * mjg:(q + 1) * mjg],
                                start=False, stop=stop,
                                perf_mode=DR, skip_group_check=True)
                        else:
                            for hb in range(2):
                                nc.tensor.matmul(
                                    t_rows[0:32, 0:mjg],
                                    rczdr[:, hb, j * 32:j * 32 + 32],
                                    ftt[:, hb, q * mjg:(q + 1) * mjg],
                                    start=False, stop=(stop and hb == 1),
                                    skip_group_check=True)

                pend = []
                for gi in range(len(GROUPS)):
                    st = stage_a(gi)
                    pend.append(st)
                    if len(pend) > 2:
                        stage_b(pend.pop(0))
                for st in pend:
                    stage_b(st)

            # ---------------- tail (t_rows pool still open) ----------------
            with tc.tile_pool(name="tail_ps", bufs=2, space="PSUM") as TPS:
                nc.vector.tensor_mul(spT[:], t_rows[0:32, :], ddctT[:])
                for vb in range(2):
                    pst2 = TPS.tile([128, 32], BF16, tag="pst2", name="pst2")
                    nc.tensor.transpose(pst2[:], spT[:, vb * 128:(vb + 1) * 128],
                                        identb[0:32, 0:32])
                    nc.vector.tensor_copy(spc[:, vb, :], pst2[:])
                up = TPS.tile([128, 2, 32], F32, tag="up", name="up")
                for ub in range(2):
                    nc.tensor.matmul(up[:, ub, :],
                                     ahbdr[:, :, ub * 128:(ub + 1) * 128],
                                     spc[:, :, :],
                                     start=True, stop=True, perf_mode=DR)
                nc.vector.tensor_mul(wt[:], up[:, :, :].rearrange("p u j -> p (u j)"),
                                     degf[:])
                nc.scalar.activation(tht[:], wt[:], AF.Tanh)
                # fin = 0.5 * tanh + dg  (the 0.5 scales the tanh OUTPUT)
                nc.vector.scalar_tensor_tensor(
                    fin[:, :, 0:32],
                    tht[:].rearrange("p (u j) -> p u j", u=2),
                    0.5, dg[:, :, :], ALU.mult, ALU.add)
        nc.sync.dma_start(out_d[:], fin[:, :, :].rearrange("p u j -> p (u j)"))

    nc.finalize()
    return nc


def _get_program():
    global _PROGRAM
    if _PROGRAM is None:
        _PROGRAM = _build_program()
    return _PROGRAM


def kernel(z, adj, W1, W2):
    global LAST_RESULTS
    from concourse.bass_utils import run_bass_kernel_spmd
    import ml_dtypes

    bf = ml_dtypes.bfloat16
    z = np.asarray(z, np.float32)
    adj = np.asarray(adj, np.float32)
    W1 = np.asarray(W1, np.float32)
    W2 = np.asarray(W2, np.float32)

    idx = np.arange(N)
    Ahat = np.maximum(adj, np.eye(N, dtype=np.float32))
    UT = (idx[:, None] < idx[None, :]).astype(np.float32)
    zt = z.T  # [128, 256]
    # column-sum prefix: CS[i, u] = sum_{v<i} Ahat[v, u]
    CSex = np.vstack([np.zeros((1, N), np.float32), np.cumsum(Ahat, axis=0)[:-1]])

    nc = _get_program()
    in_maps = []
    for c in range(NCORES):
        ii = np.arange(c, N, NCORES)
        OC = np.zeros((N, NPC), np.float32)
        OC[ii, np.arange(NPC)] = 1.0
        MC = (idx[:, None] < ii[None, :]).astype(np.float32)  # [256, 32]

        import concourse.mybir as _mybir
        f8 = _mybir.dt.np(_mybir.dt.float8e4)
        # [p, ko*512 + c]: c in 0:256 -> UT[ko*128+p], c in 256:512 -> Ahat
        b1 = np.zeros((128, 1024), f8)
        for ko in range(2):
            b1[:, ko * 512:ko * 512 + 256] = UT[ko * 128:(ko + 1) * 128].astype(f8)
            b1[:, ko * 512 + 256:ko * 512 + 512] = \
                Ahat[ko * 128:(ko + 1) * 128].astype(f8)

        b2 = np.zeros((128, 1600), bf)
        b2[:, 0:256] = zt.astype(bf)
        b2[:, 256:512] = W1[0:128].astype(bf)
        b2[0, 512:768] = W1[128].astype(bf)
        b2[:, 768:896] = W2[0:128].astype(bf)
        b2[:, 896:1024] = W2[128:256].astype(bf)
        b2[:, 1024:1280] = W2.T.astype(bf)
        b2[:, 1280:1312] = OC[0:128].astype(bf)
        b2[:, 1312:1344] = OC[128:256].astype(bf)
        b2[:, 1344:1376] = MC[0:128].astype(bf)
        b2[:, 1376:1408] = MC[128:256].astype(bf)
        b2[:, 1408:1536] = np.eye(128, dtype=np.float32).astype(bf)
        b2[:, 1536:1568] = zt[:, ii].astype(bf)

        # deg-scaled adjacency operand blocks (deg_j(w) * Ahat[w, u]) in fp8,
        # packed group-major; K=256 groups use the DoubleRow half layout
        with np.errstate(divide="ignore"):
            degM = np.where(idx[:, None] < ii[None, :],
                            CSex[ii].T ** -0.5, 0.0).astype(np.float32)  # [w, j]
        b3 = np.zeros((128, S_COLS), f8)
        for gi, (j0, g) in enumerate(GROUPS):
            mjg = 8 * (j0 + g)
            gm = g * mjg
            off = S_OFFS[gi]
            nk = 1 if mjg <= 128 else 2
            for ko in range(nk):
                w0 = ko * 128
                sz = min(mjg - w0, 128)
                blk = (Ahat[w0:w0 + sz, None, 0:mjg]
                       * degM[w0:w0 + sz, j0:j0 + g, None])  # [sz, g, mjg]
                b3[0:sz, off + ko * gm:off + ko * gm + gm] = \
                    blk.reshape(sz, gm).astype(f8)
        in_maps.append({"b1": b1, "b2": b2, "b3": b3})

    res = run_bass_kernel_spmd(nc, in_maps, list(range(NCORES)),
                               trace=TRACE, **TRACE_KW)
    LAST_RESULTS = res

    supp = np.zeros((N, N), np.float32)
    x = np.zeros((N, N), np.float32)
    for c in range(NCORES):
        ii = np.arange(c, N, NCORES)
        out_r = np.asarray(res.results[c]["outp"], np.float32).reshape(128, 2, 64)
        # supp[i_j, u] where u = ub*128 + p
        supp[ii, :] = out_r[:, :, 0:32].transpose(2, 1, 0).reshape(NPC, N)
        # x[u, i_j]
        x[:, ii] = out_r[:, :, 32:64].transpose(1, 0, 2).reshape(N, NPC)
    return (x + supp + supp.T).astype(np.float32)
